# revision 1
# baseline (speedup 1.0000x reference)
"""ConvCheb (K=3) Trainium2 kernel: batch-parallel across 8 cores.

Per core c (batch c), slab x = inputs[c] [V, F=64]:
  y1 = L @ x          (spmm1: host pre-gathered source rows, device scatter)
  z  = L @ y1         (spmm2: device dma_gather + scatter)
  out = x@(W0-W2) + y1@W1 + 2*z@W2 + bias

Scatter per 128-slot chunk is one PE matmul:
  psumT[64f, 128rows] += (gathered chunk [128slot, 64f]).T @ S[128slot, 128rows]
with S = (iota == rowadj) * val built on DVE.
Intermediates stay f-major (transposed); kernel returns outT [F, V],
un-transposed on host.
"""
import sys
for _p in ("/opt/trn_rl_repo",):
    if _p not in sys.path:
        sys.path.append(_p)
import numpy as np
import concourse.bass as bass
import concourse.bacc as bacc
import concourse.mybir as mybir
import concourse.tile as tile

dt = mybir.dt
F32 = dt.float32


def _pad128(n):
    return max((n + 127) & ~127, 128)


def build_graph_plan(rows, cols, vals, V, block=128):
    """Per-parity chunk tables for one spmm pass (slots sorted by dest block,
    padded per block to 128-multiples; >=1 chunk per block per parity)."""
    nb = V // block
    order = np.argsort(rows, kind="stable")
    rows_s, cols_s, vals_s = rows[order], cols[order], vals[order]
    out = {}
    for parity in (0, 1):
        sel = (cols_s & 1) == parity
        r, c, v = rows_s[sel], cols_s[sel], vals_s[sel]
        chunk_cols, chunk_rowadj, chunk_val, nchunks = [], [], [], []
        for b in range(nb):
            lo = np.searchsorted(r, b * block, "left")
            hi = np.searchsorted(r, (b + 1) * block - 1, "right")
            rb, cb, vb = r[lo:hi], c[lo:hi], v[lo:hi]
            n = _pad128(len(rb))
            pad = n - len(rb)
            rb = np.concatenate([rb - b * block, np.zeros(pad, rb.dtype)])
            cb = np.concatenate([cb, np.zeros(pad, cb.dtype)])
            vb = np.concatenate([vb, np.zeros(pad, np.float32)])
            nchunks.append(n // 128)
            chunk_cols.append(cb)
            chunk_rowadj.append(rb)
            chunk_val.append(vb)
        out[parity] = dict(
            cols=np.concatenate(chunk_cols).astype(np.int64),
            rowadj=np.concatenate(chunk_rowadj).astype(np.float32),
            val=np.concatenate(chunk_val).astype(np.float32),
            nchunks=np.array([int(x) for x in nchunks]),
        )
    return out


def merge_parities(plan):
    """For spmm1 (host-gathered) the parity split is unnecessary: merge into
    a single region with per-block counts."""
    p0, p1 = plan[0], plan[1]
    nb = len(p0["nchunks"])
    cols, rowadj, val, nchunks = [], [], [], []
    o0 = o1 = 0
    for b in range(nb):
        n0, n1 = p0["nchunks"][b] * 128, p1["nchunks"][b] * 128
        cols += [p0["cols"][o0:o0 + n0], p1["cols"][o1:o1 + n1]]
        rowadj += [p0["rowadj"][o0:o0 + n0], p1["rowadj"][o1:o1 + n1]]
        val += [p0["val"][o0:o0 + n0], p1["val"][o1:o1 + n1]]
        nchunks.append((n0 + n1) // 128)
        o0 += n0
        o1 += n1
    return dict(cols=np.concatenate(cols), rowadj=np.concatenate(rowadj),
                val=np.concatenate(val), nchunks=np.array(nchunks))


def slots_to_chunk_layout(arr):
    """[nslots(, d)] -> [128, nchunks(, d)]: slot j -> [j%128, j//128]."""
    n = arr.shape[0] // 128
    a = arr.reshape(n, 128, *arr.shape[1:])
    return np.ascontiguousarray(np.moveaxis(a, 1, 0))


def wrap_idx16(idx):
    """dma_gather idx layout [128, n/16] int16: idx j at [j%16, j//16],
    replicated across the 8 groups of 16 partitions."""
    n = len(idx)
    assert n % 128 == 0
    w = np.zeros((16, n // 16), np.int16)
    for p in range(16):
        w[p, :] = idx[p::16]
    return np.ascontiguousarray(np.tile(w, (8, 1)))


def make_sv(plan_p):
    ra = slots_to_chunk_layout(plan_p["rowadj"])
    va = slots_to_chunk_layout(plan_p["val"])
    return np.ascontiguousarray(np.stack([ra, va], axis=-1).astype(np.float32))


def build_kernel(nc, V, F, C1, C2e, C2o, n1, n2e, n2o, gpiece=4096, phases='12b'):
    nb = V // 128
    Vh = V // 2
    assert F == 64
    GB = min(8, nb)
    CP = gpiece // 128  # chunks per piece

    g1_d = nc.dram_tensor("g1", [128, C1, F], F32, kind="ExternalInput")
    sv1_d = nc.dram_tensor("sv1", [128, C1, 2], F32, kind="ExternalInput")
    sv2e_d = nc.dram_tensor("sv2e", [128, C2e, 2], F32, kind="ExternalInput")
    sv2o_d = nc.dram_tensor("sv2o", [128, C2o, 2], F32, kind="ExternalInput")
    idx2e_d = nc.dram_tensor("idx2e", [128, C2e * 8], dt.int16, kind="ExternalInput")
    idx2o_d = nc.dram_tensor("idx2o", [128, C2o * 8], dt.int16, kind="ExternalInput")
    x0t_d = nc.dram_tensor("x0t", [F, V], F32, kind="ExternalInput")
    w_d = nc.dram_tensor("w3", [2 * F, 3, F], F32, kind="ExternalInput")  # [f(x2 halves), term, fout]
    bias_d = nc.dram_tensor("bias", [F, 1], F32, kind="ExternalInput")
    iota_d = nc.dram_tensor("iota", [128, 128], F32, kind="ExternalInput")
    ident_d = nc.dram_tensor("ident", [128, 64], F32, kind="ExternalInput")
    y1_d = nc.dram_tensor("y1", [V, F], F32)  # internal
    outt_d = nc.dram_tensor("outt", [F, V], F32, kind="ExternalOutput")

    def pieces(C):
        return [(p * CP, min(CP, C - p * CP)) for p in range((C + CP - 1) // CP)]

    with tile.TileContext(nc) as tc:
        with (
            tc.tile_pool(name="const", bufs=1) as cpool,
            tc.tile_pool(name="ybig", bufs=1) as ypool,
            tc.tile_pool(name="g1p", bufs=3) as g1pool,
            tc.tile_pool(name="g2e", bufs=3) as g2epool,
            tc.tile_pool(name="g2o", bufs=3) as g2opool,
            tc.tile_pool(name="svp", bufs=4) as svpool,
            tc.tile_pool(name="idxp", bufs=4) as idxpool,
            tc.tile_pool(name="sp", bufs=6) as spool,
            tc.tile_pool(name="ztp", bufs=2) as zpool,
            tc.tile_pool(name="xtp", bufs=2) as xtpool,
            tc.tile_pool(name="stg", bufs=3) as stgpool,
            tc.tile_pool(name="psA", bufs=2, space="PSUM") as psA,
            tc.tile_pool(name="psT", bufs=2, space="PSUM") as psT,
            tc.tile_pool(name="psG", bufs=2, space="PSUM") as psG,
        ):
            iota_t = cpool.tile([128, 128], F32)
            nc.sync.dma_start(iota_t[:], iota_d.ap())
            ident_t = cpool.tile([128, 64], F32)
            nc.sync.dma_start(ident_t[:], ident_d.ap())
            w_t = cpool.tile([2 * F, 3, F], F32)
            nc.sync.dma_start(w_t[:], w_d.ap())
            bias_t = cpool.tile([F, 1], F32)
            nc.sync.dma_start(bias_t[:], bias_d.ap())

            y1t_t = ypool.tile([128, Vh], F32)  # p = f + 64*(v >= Vh)

            # ---------- PHASE 1 ----------
            g1_tiles, sv1_tiles = [], []
            p1list = pieces(C1)

            def emit_p1(p):
                c0, w = p1list[p]
                g1_t = g1pool.tile([128, CP, F], F32, tag="g1")
                nc.sync.dma_start(g1_t[:, 0:w, :], g1_d.ap()[:, c0:c0 + w, :])
                sv1_t = svpool.tile([128, CP, 2], F32, tag="sv1")
                nc.sync.dma_start(sv1_t[:, 0:w, :], sv1_d.ap()[:, c0:c0 + w, :])
                g1_tiles.append(g1_t)
                sv1_tiles.append(sv1_t)

            def scatter_chunk(ps, g_tiles, sv_tiles, cglob, first, last):
                p, cip = divmod(cglob, CP)
                s_t = spool.tile([128, 128], F32, tag="S")
                nc.vector.tensor_scalar(
                    s_t[:], iota_t[:],
                    sv_tiles[p][:, cip, 0:1], sv_tiles[p][:, cip, 1:2],
                    op0=mybir.AluOpType.is_equal, op1=mybir.AluOpType.mult,
                )
                nc.tensor.matmul(ps[:], g_tiles[p][:, cip, :], s_t[:],
                                 start=first, stop=last)

            cglob = 0
            ystage = None
            for b in (range(nb) if '1' in phases else []):
                ps = psA.tile([64, 128], F32, tag="psA")
                for j in range(n1[b]):
                    while cglob // CP >= len(g1_tiles):
                        emit_p1(len(g1_tiles))
                    scatter_chunk(ps, g1_tiles, sv1_tiles, cglob,
                                  j == 0, j == n1[b] - 1)
                    cglob += 1
                half, off = divmod(b * 128, Vh)
                nc.scalar.copy(y1t_t[64 * half:64 * half + 64, off:off + 128], ps[:])
                pt = psT.tile([128, 64], F32, tag="psT")
                nc.tensor.transpose(
                    pt[:], y1t_t[64 * half:64 * half + 64, off:off + 128],
                    ident_t[64 * half:64 * half + 64, :])
                if b % GB == 0:
                    ystage = stgpool.tile([128, GB, 64], F32, tag="yst")
                nc.scalar.copy(ystage[:, b % GB, :], pt[:])
                if b % GB == GB - 1:
                    g = b // GB
                    dst = y1_d.ap().rearrange("(g e p) f -> g p e f", e=GB, p=128)
                    nc.sync.dma_start(dst[g], ystage[:])

            # ---------- PHASE boundary ----------
            if 'b' in phases:
                tc.strict_bb_all_engine_barrier()
            if '2' not in phases:
                # debug: dump y1t to outt instead of running phase 2
                W = 512
                ot = outt_d.ap().rearrange("f (h v) -> h f v", h=2)
                for g in range(Vh // W):
                    st = stgpool.tile([128, W], F32, tag="dbg")
                    nc.vector.tensor_copy(st[:], y1t_t[:, g * W:(g + 1) * W])
                    nc.sync.dma_start(ot[0][:, g * W:(g + 1) * W], st[0:64, :])
                    nc.sync.dma_start(ot[1][:, g * W:(g + 1) * W], st[64:128, :])
                return

            # ---------- PHASE 2 ----------
            y1v = y1_d.ap().rearrange("(a b) f -> a b f", b=2)

            class GatherStream:
                def __init__(self, C2, idx_d, sv_d, src_ap, pool, tag):
                    self.plist = pieces(C2)
                    self.idx_d, self.sv_d, self.src_ap = idx_d, sv_d, src_ap
                    self.pool, self.tag = pool, tag
                    self.g_tiles, self.sv_tiles = [], []

                def ensure(self, cglob):
                    while cglob // CP >= len(self.g_tiles):
                        p = len(self.g_tiles)
                        c0, w = self.plist[p]
                        it = idxpool.tile([128, CP * 8], dt.int16, tag="idx" + self.tag)
                        nc.sync.dma_start(it[:, 0:w * 8],
                                          self.idx_d.ap()[:, c0 * 8:(c0 + w) * 8])
                        sv_t = svpool.tile([128, CP, 2], F32, tag="sv" + self.tag)
                        nc.sync.dma_start(sv_t[:, 0:w, :],
                                          self.sv_d.ap()[:, c0:c0 + w, :])
                        gt = self.pool.tile([128, CP, F], F32, tag=self.tag)
                        if 'p' in phases:
                            nc.sync.dma_start(
                                gt[:, 0:w, :],
                                y1_d.ap().rearrange("(a p) f -> p a f", p=128)[:, 0:w, :])
                        else:
                            nc.gpsimd.dma_gather(
                                gt[:, 0:w, :], self.src_ap, it[:, 0:w * 8],
                                num_idxs=w * 128, num_idxs_reg=w * 128,
                                elem_size=F, elem_step=2 * F, single_packet=False,
                            )
                        self.g_tiles.append(gt)
                        self.sv_tiles.append(sv_t)

            se = GatherStream(C2e, idx2e_d, sv2e_d, y1v[:, 0, :], g2epool, "g2e")
            so = GatherStream(C2o, idx2o_d, sv2o_d, y1v[:, 1, :], g2opool, "g2o")

            ce = co = 0
            ostage = None
            xt_t = None
            for b in range(nb):
                ps = psA.tile([64, 128], F32, tag="psA")
                tot = n2e[b] + n2o[b]
                jj = 0
                for j in range(n2e[b]):
                    se.ensure(ce)
                    scatter_chunk(ps, se.g_tiles, se.sv_tiles, ce, jj == 0, jj == tot - 1)
                    ce += 1
                    jj += 1
                for j in range(n2o[b]):
                    so.ensure(co)
                    scatter_chunk(ps, so.g_tiles, so.sv_tiles, co, jj == 0, jj == tot - 1)
                    co += 1
                    jj += 1
                z_t = zpool.tile([64, 128], F32, tag="zT")
                nc.scalar.copy(z_t[:], ps[:])
                if b % GB == 0:
                    xt_t = xtpool.tile([F, GB, 128], F32, tag="xt")
                    nc.sync.dma_start(
                        xt_t[:], x0t_d.ap()[:, b * 128:(b + GB) * 128]
                        .rearrange("f (e v) -> f e v", e=GB))
                    ostage = stgpool.tile([F, GB, 128], F32, tag="ost")
                half, off = divmod(b * 128, Vh)
                if 'g' in phases:
                    nc.vector.tensor_copy(ostage[:, b % GB, :], z_t[:])
                else:
                    pg = psG.tile([64, 128], F32, tag="psG")
                    nc.tensor.matmul(pg[:], w_t[0:64, 0, :], xt_t[:, b % GB, :], start=True, stop=False)
                    nc.tensor.matmul(pg[:], w_t[0:64, 2, :], z_t[:], start=False, stop=True)
                    pg2 = psG.tile([64, 128], F32, tag="psG2")
                    nc.tensor.matmul(pg2[:], w_t[64 * half:64 * half + 64, 1, :],
                                     y1t_t[64 * half:64 * half + 64, off:off + 128],
                                     start=True, stop=True)
                    nc.scalar.activation(
                        ostage[:, b % GB, :], pg[:],
                        mybir.ActivationFunctionType.Identity, bias=bias_t[:])
                    nc.vector.tensor_tensor(
                        ostage[:, b % GB, :], ostage[:, b % GB, :], pg2[:],
                        op=mybir.AluOpType.add)
                if b % GB == GB - 1:
                    g = b // GB
                    nc.sync.dma_start(
                        outt_d.ap()[:, g * GB * 128:(g + 1) * GB * 128],
                        ostage[:].rearrange("f e v -> f (e v)"))


def host_prepare(rows, cols, vals, V, F, gpiece=4096):
    """Graph-dependent (core-independent) host tables."""
    plan2 = build_graph_plan(rows, cols, vals, V)
    plan1 = merge_parities(plan2)
    C1 = len(plan1["cols"]) // 128
    C2e = len(plan2[0]["cols"]) // 128
    C2o = len(plan2[1]["cols"]) // 128
    tabs = dict(
        C1=C1, C2e=C2e, C2o=C2o,
        n1=plan1["nchunks"], n2e=plan2[0]["nchunks"], n2o=plan2[1]["nchunks"],
        sv1=make_sv(plan1), sv2e=make_sv(plan2[0]), sv2o=make_sv(plan2[1]),
        idx2e=wrap_idx16((plan2[0]["cols"] >> 1).astype(np.int16)),
        idx2o=wrap_idx16((plan2[1]["cols"] >> 1).astype(np.int16)),
        cols1=plan1["cols"],
        iota=np.tile(np.arange(128, dtype=np.float32), (128, 1)),
        ident=np.tile(np.eye(64, dtype=np.float32), (2, 1)),
    )
    return tabs


def make_in_map(tabs, x_slab, weight, bias):
    """Per-core inputs. x_slab [V, F] f32."""
    F = x_slab.shape[1]
    g1 = slots_to_chunk_layout(x_slab[tabs["cols1"], :])
    wa = weight[:, 0, :] - weight[:, 2, :]
    wb = weight[:, 1, :]
    wc = 2.0 * weight[:, 2, :]
    w3 = np.ascontiguousarray(np.tile(
        np.stack([wa, wb, wc], axis=1).astype(np.float32), (2, 1, 1)))
    return {
        "g1": np.ascontiguousarray(g1),
        "sv1": tabs["sv1"], "sv2e": tabs["sv2e"], "sv2o": tabs["sv2o"],
        "idx2e": tabs["idx2e"], "idx2o": tabs["idx2o"],
        "x0t": np.ascontiguousarray(x_slab.T),
        "w3": w3,
        "bias": bias.reshape(F, 1).astype(np.float32),
        "iota": tabs["iota"], "ident": tabs["ident"],
    }


# ----------------------------------------------------------------------------
# Self-contained entry point. Hardcoded problem dims (nn_ConvCheb):
#   inputs [8, 49152, 64] f32, weight [64, 3, 64], bias [64],
#   lap_rows/cols/vals [393216] (COO Laplacian, replicated per core).
# Sharding: batch-parallel, core c owns batch c (zero cross-core traffic).
# ----------------------------------------------------------------------------
_KERNEL_CACHE = {}


def _get_compiled(tabs, V, F):
    key = "k"
    if key not in _KERNEL_CACHE:
        nc = bacc.Bacc("TRN2", target_bir_lowering=False, debug=False,
                       num_devices=8)
        build_kernel(nc, V, F, tabs["C1"], tabs["C2e"], tabs["C2o"],
                     tabs["n1"], tabs["n2e"], tabs["n2o"], gpiece=4096)
        nc.compile()
        _KERNEL_CACHE[key] = nc
    return _KERNEL_CACHE[key]


def kernel(inputs, weight, bias, lap_rows, lap_cols, lap_vals):
    from concourse.bass_utils import run_bass_kernel_spmd

    B, V, F = inputs.shape
    assert (B, V, F) == (8, 49152, 64)
    rows = np.asarray(lap_rows).astype(np.int64)
    cols = np.asarray(lap_cols).astype(np.int64)
    vals = np.asarray(lap_vals).astype(np.float32)
    inputs = np.asarray(inputs, dtype=np.float32)
    weight = np.asarray(weight, dtype=np.float32)
    bias = np.asarray(bias, dtype=np.float32)

    tabs = host_prepare(rows, cols, vals, V, F)
    nc = _get_compiled(tabs, V, F)

    in_maps = [make_in_map(tabs, inputs[c], weight, bias) for c in range(B)]
    res = run_bass_kernel_spmd(nc, in_maps, list(range(B)))
    out = np.stack([np.ascontiguousarray(res.results[c]["outt"].T)
                    for c in range(B)])
    return out.astype(np.float32)



# revision 8
# speedup vs baseline: 1.1895x; 1.1895x over previous
"""ConvCheb (K=3) Trainium2 kernel: batch-parallel across 8 cores, v2.

Per core c (batch c), slab x = inputs[c] [V, F=64]:
  y1 = L @ x          (pass 1: host pre-gathered bf16 slots, streamed S)
  z  = L @ y1         (pass 2: device dma_gather of bf16 row-pairs, streamed S)
  out = x@(W0-W2) + y1@W1 + 2*z@W2 + bias

Key differences from v1:
  - Scatter matrices S (one-hot rows scaled by lap_vals) are precomputed on
    the host and STREAMED from HBM in bf16 instead of built per-chunk on DVE
    (tensor_scalar with vector scalars measured 1185ns/chunk -> DVE 93% busy).
  - All matmul data is bf16 (fp32 PSUM accumulation).
  - Dest blocks are 64 rows wide -> S is [128, 64] (16KB bf16) per chunk.
  - Pass-2 gathers bf16 row PAIRS (256B elements, idx = col>>1 fits int16)
    from a plain [V, 64] bf16 y1; chunks are sorted by (dest block, col
    parity) so each chunk reads one 64-wide plane of the gathered pair.
  - Gathers round-robin across 4 SWDGE queues (num_swdge_queues=4) so
    descriptor generation uses all 4 Q7 core-pairs concurrently.
"""
import sys
for _p in ("/opt/trn_rl_repo",):
    if _p not in sys.path:
        sys.path.append(_p)
import numpy as np
import ml_dtypes
import concourse.bass as bass
import concourse.bacc as bacc
import concourse.mybir as mybir
import concourse.tile as tile

dt = mybir.dt
F32 = dt.float32
BF16 = dt.bfloat16
NPBF16 = ml_dtypes.bfloat16

V = 49152
F = 64
W = 64            # dest-block width (rows per psum tile)
NB = V // W       # 768 dest blocks
VH = V // 2
CP = 32           # chunks per streamed piece
NQ = 1            # SWDGE queues (bisect: was 4)
import os
PHASES = os.environ.get("KPHASES", "12")   # '1' = pass-1 only (debug bisect)
GATHER = os.environ.get("KGATHER", "1") == "1"  # 0: replace gather with dma
GB = 8            # blocks64 staged per y1 DMA
GG = 8            # block128 groups staged per out DMA


def _pad128(n):
    return max((n + 127) & ~127, 128)


def build_plan(rows, cols, vals, split_parity):
    """Chunk tables: slots sorted by dest block64 (and col parity when
    split_parity), padded per group to 128-multiples (>=128 per block).

    Returns dict with per-slot arrays (cols, local row, val) and per-block
    chunk counts. For split_parity, each block's chunks are even-cols chunks
    followed by odd-cols chunks (counts in nch_e / nch_o)."""
    order = np.argsort(rows, kind="stable")
    rows_s, cols_s, vals_s = rows[order], cols[order], vals[order]
    blk = rows_s // W
    out_cols, out_radj, out_val, parities = [], [], [], []
    nch_e = np.zeros(NB, np.int32)
    nch_o = np.zeros(NB, np.int32)
    lo_all = np.searchsorted(blk, np.arange(NB), "left")
    hi_all = np.searchsorted(blk, np.arange(NB), "right")
    for b in range(NB):
        lo, hi = lo_all[b], hi_all[b]
        rb, cb, vb = rows_s[lo:hi] - b * W, cols_s[lo:hi], vals_s[lo:hi]
        groups = []
        if split_parity:
            sel = (cb & 1) == 0
            groups.append((rb[sel], cb[sel], vb[sel], 0))
            groups.append((rb[~sel], cb[~sel], vb[~sel], 1))
        else:
            groups.append((rb, cb, vb, 0))
        for gr, gc, gv, par in groups:
            n = _pad128(len(gr))
            pad = n - len(gr)
            out_radj.append(np.concatenate([gr, np.zeros(pad, gr.dtype)]))
            out_cols.append(np.concatenate([gc, np.full(pad, par, gc.dtype)]))
            out_val.append(np.concatenate([gv, np.zeros(pad, np.float32)]))
            parities.append(np.full(n // 128, par, np.int32))
            if par == 0:
                nch_e[b] = n // 128
            else:
                nch_o[b] = n // 128
    return dict(
        cols=np.concatenate(out_cols),
        radj=np.concatenate(out_radj).astype(np.int32),
        val=np.concatenate(out_val).astype(np.float32),
        nch_e=nch_e, nch_o=nch_o,
        parity=np.concatenate(parities),
    )


def slots_to_chunk_layout(arr):
    """[nslots(, d)] -> [128, nchunks(, d)]: slot j -> [j%128, j//128]."""
    n = arr.shape[0] // 128
    a = arr.reshape(n, 128, *arr.shape[1:])
    return np.ascontiguousarray(np.moveaxis(a, 1, 0))


def wrap_idx16(idx):
    """dma_gather idx layout [128, n/16] int16: idx j at [j%16, j//16],
    replicated across the 8 groups of 16 partitions."""
    n = len(idx)
    assert n % 128 == 0
    w = np.zeros((16, n // 16), np.int16)
    for p in range(16):
        w[p, :] = idx[p::16]
    return np.ascontiguousarray(np.tile(w, (8, 1)))


def make_s(plan):
    """Dense scatter matrices [128, C, W] bf16: S[s, c, r] = val at slot
    (c*128+s) if its local dest row == r else 0."""
    nslots = len(plan["val"])
    C = nslots // 128
    s = np.zeros((C, 128, W), np.float32)
    ci = np.arange(nslots) // 128
    si = np.arange(nslots) % 128
    s[ci, si, plan["radj"]] = plan["val"]
    return np.ascontiguousarray(np.moveaxis(s, 1, 0)).astype(NPBF16)


def build_kernel(nc, C1, C2, n1, n2e, n2o, par2):
    NB128 = V // 128
    g1_d = nc.dram_tensor("g1", [128, C1, F], BF16, kind="ExternalInput")
    s1_d = nc.dram_tensor("s1", [128, C1, W], BF16, kind="ExternalInput")
    s2_d = nc.dram_tensor("s2", [128, C2, W], BF16, kind="ExternalInput")
    idx2_d = nc.dram_tensor("idx2", [128, C2 * 8], dt.int16, kind="ExternalInput")
    x0t_d = nc.dram_tensor("x0t", [F, V], BF16, kind="ExternalInput")
    w_d = nc.dram_tensor("w3", [2 * F, 3, F], BF16, kind="ExternalInput")
    bias_d = nc.dram_tensor("bias", [F, 1], F32, kind="ExternalInput")
    ident_d = nc.dram_tensor("ident", [128, 64], F32, kind="ExternalInput")
    y1_d = nc.dram_tensor("y1", [V, F], BF16)  # internal, row-major
    outt_d = nc.dram_tensor("outt", [F, V], F32, kind="ExternalOutput")

    def pieces(C):
        return [(p * CP, min(CP, C - p * CP)) for p in range((C + CP - 1) // CP)]

    with tile.TileContext(nc) as tc:
        with (
            tc.tile_pool(name="const", bufs=1) as cpool,
            tc.tile_pool(name="ybig", bufs=1) as ypool,
            tc.tile_pool(name="g1p", bufs=3) as g1pool,
            tc.tile_pool(name="s1p", bufs=3) as s1pool,
            tc.tile_pool(name="g2p", bufs=4) as g2pool,
            tc.tile_pool(name="s2p", bufs=4) as s2pool,
            tc.tile_pool(name="idxp", bufs=4) as idxpool,
            tc.tile_pool(name="ztp", bufs=2) as zpool,
            tc.tile_pool(name="xtp", bufs=2) as xtpool,
            tc.tile_pool(name="stg", bufs=3) as stgpool,
            tc.tile_pool(name="t32", bufs=3) as t32pool,
            tc.tile_pool(name="psA", bufs=2, space="PSUM") as psA,
            tc.tile_pool(name="psT", bufs=2, space="PSUM") as psT,
            tc.tile_pool(name="psB", bufs=2, space="PSUM") as psB,
            tc.tile_pool(name="psG", bufs=1, space="PSUM") as psG,
        ):
            ident_t = cpool.tile([128, 64], F32)
            nc.sync.dma_start(ident_t[:], ident_d.ap())
            w_t = cpool.tile([2 * F, 3, F], BF16)
            nc.sync.dma_start(w_t[:], w_d.ap())
            bias_t = cpool.tile([F, 1], F32)
            nc.sync.dma_start(bias_t[:], bias_d.ap())

            y1t_t = ypool.tile([128, VH], BF16)  # p = f + 64*(v >= VH)

            # ---------- PASS 1 ----------
            g1_tiles, s1_tiles = [], []
            p1list = pieces(C1)

            def emit_p1(p):
                c0, w = p1list[p]
                g1_t = g1pool.tile([128, CP, F], BF16, tag="g1")
                nc.sync.dma_start(g1_t[:, 0:w, :], g1_d.ap()[:, c0:c0 + w, :])
                s1_t = s1pool.tile([128, CP, W], BF16, tag="s1")
                nc.sync.dma_start(s1_t[:, 0:w, :], s1_d.ap()[:, c0:c0 + w, :])
                g1_tiles.append(g1_t)
                s1_tiles.append(s1_t)

            cglob = 0
            ystage = None
            for b in range(NB):
                ps = psA.tile([64, W], F32, tag="psA")
                for j in range(n1[b]):
                    while cglob // CP >= len(g1_tiles):
                        emit_p1(len(g1_tiles))
                    p, cip = divmod(cglob, CP)
                    nc.tensor.matmul(ps[:], g1_tiles[p][:, cip, :],
                                     s1_tiles[p][:, cip, :],
                                     start=(j == 0), stop=(j == n1[b] - 1))
                    cglob += 1
                half, off = divmod(b * W, VH)
                ysl = y1t_t[64 * half:64 * half + 64, off:off + W]
                nc.vector.tensor_copy(ysl, ps[:])
                t32 = t32pool.tile([64, W], F32, tag="t32")
                nc.scalar.copy(t32[:], ps[:])
                pt = psT.tile([W, 64], F32, tag="psT")
                nc.tensor.transpose(pt[:], t32[:], ident_t[0:64, :])
                if b % GB == 0:
                    ystage = stgpool.tile([W, GB, 64], BF16, tag="yst")
                nc.scalar.copy(ystage[:, b % GB, :], pt[:])
                if b % GB == GB - 1:
                    g = b // GB
                    dst = y1_d.ap().rearrange("(g e p) f -> g p e f", e=GB, p=W)
                    nc.sync.dma_start(dst[g], ystage[:])

            # ---------- PHASE boundary (debug bisect) ----------
            if '2' not in PHASES:
                ot = outt_d.ap().rearrange("f (h v) -> h f v", h=2)
                for g in range(VH // 512):
                    st = stgpool.tile([128, 512], F32, tag="dbg")
                    nc.vector.tensor_copy(st[:], y1t_t[:, g * 512:(g + 1) * 512])
                    nc.sync.dma_start(ot[0][:, g * 512:(g + 1) * 512], st[0:64, :])
                    nc.sync.dma_start(ot[1][:, g * 512:(g + 1) * 512], st[64:128, :])
                return

            # ---------- PASS 2 ----------
            y1pair = y1_d.ap().rearrange("(p two) f -> p (two f)", two=2)
            p2list = pieces(C2)
            g2_tiles, s2_tiles = [], []

            def emit_p2(p):
                c0, w = p2list[p]
                it = idxpool.tile([128, CP * 8], dt.int16, tag="idx2")
                nc.sync.dma_start(it[:, 0:w * 8],
                                  idx2_d.ap()[:, c0 * 8:(c0 + w) * 8])
                s2_t = s2pool.tile([128, CP, W], BF16, tag="s2")
                nc.sync.dma_start(s2_t[:, 0:w, :], s2_d.ap()[:, c0:c0 + w, :])
                gt = g2pool.tile([128, CP, 2 * F], BF16, tag="g2")
                if not GATHER:
                    nc.sync.dma_start(
                        gt[:, 0:w, :],
                        y1pair.rearrange("(a p) f -> p a f", p=128)[:, 0:w, :])
                    g2_tiles.append(gt)
                    s2_tiles.append(s2_t)
                    return
                nc.gpsimd.dma_gather(
                    gt[:, 0:w, :], y1pair, it[:, 0:w * 8],
                    num_idxs=w * 128, num_idxs_reg=w * 128,
                    elem_size=2 * F, single_packet=False,
                    queue_num=p % NQ,
                )
                g2_tiles.append(gt)
                s2_tiles.append(s2_t)

            cglob = 0
            ostage = None
            xt_t = None
            zt_t = None
            for b in range(NB):
                ps2 = psB.tile([64, W], F32, tag="psB")
                tot = n2e[b] + n2o[b]
                for j in range(tot):
                    while cglob // CP >= len(g2_tiles):
                        emit_p2(len(g2_tiles))
                    p, cip = divmod(cglob, CP)
                    par = par2[cglob]
                    nc.tensor.matmul(
                        ps2[:], g2_tiles[p][:, cip, 64 * par:64 * par + 64],
                        s2_tiles[p][:, cip, :],
                        start=(j == 0), stop=(j == tot - 1))
                    cglob += 1
                if b % 2 == 0:
                    zt_t = zpool.tile([64, 2, W], BF16, tag="zt")
                nc.vector.tensor_copy(zt_t[:, b % 2, :], ps2[:])
                if b % 2 == 1:
                    bb = b // 2  # block128 index
                    half = (bb * 128) // VH
                    off = (bb * 128) % VH
                    if bb % GG == 0:
                        xt_t = xtpool.tile([F, GG, 128], BF16, tag="xt")
                        nc.sync.dma_start(
                            xt_t[:], x0t_d.ap()[:, bb * 128:(bb + GG) * 128]
                            .rearrange("f (e v) -> f e v", e=GG))
                        ostage = stgpool.tile([F, GG, 128], F32, tag="ost")
                    pg = psG.tile([64, 128], F32, tag="psG")
                    nc.tensor.matmul(pg[:], w_t[0:64, 0, :],
                                     xt_t[:, bb % GG, :], start=True, stop=False)
                    nc.tensor.matmul(pg[:], w_t[0:64, 2, :],
                                     zt_t[:].rearrange("f two w -> f (two w)"),
                                     start=False, stop=True)
                    pg2 = psG.tile([64, 128], F32, tag="psG2")
                    nc.tensor.matmul(pg2[:], w_t[64 * half:64 * half + 64, 1, :],
                                     y1t_t[64 * half:64 * half + 64, off:off + 128],
                                     start=True, stop=True)
                    nc.scalar.activation(
                        ostage[:, bb % GG, :], pg[:],
                        mybir.ActivationFunctionType.Identity, bias=bias_t[:])
                    nc.vector.tensor_tensor(
                        ostage[:, bb % GG, :], ostage[:, bb % GG, :], pg2[:],
                        op=mybir.AluOpType.add)
                    if bb % GG == GG - 1:
                        g = bb // GG
                        nc.sync.dma_start(
                            outt_d.ap()[:, g * GG * 128:(g + 1) * GG * 128],
                            ostage[:].rearrange("f e v -> f (e v)"))


def host_prepare(rows, cols, vals):
    """Graph-dependent (core-independent) host tables."""
    plan1 = build_plan(rows, cols, vals, split_parity=False)
    plan2 = build_plan(rows, cols, vals, split_parity=True)
    C1 = len(plan1["val"]) // 128
    C2 = len(plan2["val"]) // 128
    tabs = dict(
        C1=C1, C2=C2,
        n1=plan1["nch_e"], n2e=plan2["nch_e"], n2o=plan2["nch_o"],
        par2=plan2["parity"],
        s1=make_s(plan1), s2=make_s(plan2),
        idx2=wrap_idx16((plan2["cols"] >> 1).astype(np.int16)),
        cols1=plan1["cols"],
        ident=np.tile(np.eye(64, dtype=np.float32), (2, 1)),
    )
    return tabs


def make_in_map(tabs, x_slab, weight, bias):
    """Per-core inputs. x_slab [V, F] f32."""
    g1 = slots_to_chunk_layout(x_slab[tabs["cols1"], :].astype(NPBF16))
    wa = weight[:, 0, :] - weight[:, 2, :]
    wb = weight[:, 1, :]
    wc = 2.0 * weight[:, 2, :]
    w3 = np.ascontiguousarray(np.tile(
        np.stack([wa, wb, wc], axis=1), (2, 1, 1))).astype(NPBF16)
    return {
        "g1": np.ascontiguousarray(g1),
        "s1": tabs["s1"], "s2": tabs["s2"], "idx2": tabs["idx2"],
        "x0t": np.ascontiguousarray(x_slab.T).astype(NPBF16),
        "w3": w3,
        "bias": bias.reshape(F, 1).astype(np.float32),
        "ident": tabs["ident"],
    }


_KERNEL_CACHE = {}


def _get_compiled(tabs):
    key = "k"
    if key not in _KERNEL_CACHE:
        nc = bacc.Bacc("TRN2", target_bir_lowering=False, debug=False,
                       num_devices=8, num_swdge_queues=NQ)
        build_kernel(nc, tabs["C1"], tabs["C2"],
                     tabs["n1"], tabs["n2e"], tabs["n2o"], tabs["par2"])
        nc.compile()
        _KERNEL_CACHE[key] = nc
    return _KERNEL_CACHE[key]


def kernel(inputs, weight, bias, lap_rows, lap_cols, lap_vals):
    from concourse.bass_utils import run_bass_kernel_spmd

    B, Vi, Fi = inputs.shape
    assert (B, Vi, Fi) == (8, V, F)
    rows = np.asarray(lap_rows).astype(np.int64)
    cols = np.asarray(lap_cols).astype(np.int64)
    vals = np.asarray(lap_vals).astype(np.float32)
    inputs = np.asarray(inputs, dtype=np.float32)
    weight = np.asarray(weight, dtype=np.float32)
    bias = np.asarray(bias, dtype=np.float32)

    tabs = host_prepare(rows, cols, vals)
    nc = _get_compiled(tabs)

    in_maps = [make_in_map(tabs, inputs[c], weight, bias) for c in range(B)]
    res = run_bass_kernel_spmd(nc, in_maps, list(range(B)))
    out = np.stack([np.ascontiguousarray(res.results[c]["outt"].T)
                    for c in range(B)])
    return out.astype(np.float32)


# revision 9
# speedup vs baseline: 1.8646x; 1.5675x over previous
"""ConvCheb (K=3) Trainium2 kernel: batch-parallel across 8 cores, v2.

Per core c (batch c), slab x = inputs[c] [V, F=64]:
  y1 = L @ x          (pass 1: host pre-gathered bf16 slots, streamed S)
  z  = L @ y1         (pass 2: device dma_gather of bf16 row-pairs, streamed S)
  out = x@(W0-W2) + y1@W1 + 2*z@W2 + bias

Key differences from v1:
  - Scatter matrices S (one-hot rows scaled by lap_vals) are precomputed on
    the host and STREAMED from HBM in bf16 instead of built per-chunk on DVE
    (tensor_scalar with vector scalars measured 1185ns/chunk -> DVE 93% busy).
  - All matmul data is bf16 (fp32 PSUM accumulation).
  - Dest blocks are 64 rows wide -> S is [128, 64] (16KB bf16) per chunk.
  - Pass-2 gathers bf16 row PAIRS (256B elements, idx = col>>1 fits int16)
    from a plain [V, 64] bf16 y1; chunks are sorted by (dest block, col
    parity) so each chunk reads one 64-wide plane of the gathered pair.
  - Gathers round-robin across 4 SWDGE queues (num_swdge_queues=4) so
    descriptor generation uses all 4 Q7 core-pairs concurrently.
"""
import sys
for _p in ("/opt/trn_rl_repo",):
    if _p not in sys.path:
        sys.path.append(_p)
import numpy as np
import ml_dtypes
import concourse.bass as bass
import concourse.bacc as bacc
import concourse.mybir as mybir
import concourse.tile as tile

dt = mybir.dt
F32 = dt.float32
BF16 = dt.bfloat16
NPBF16 = ml_dtypes.bfloat16

V = 49152
F = 64
W = 64            # dest-block width (rows per psum tile)
NB = V // W       # 768 dest blocks
VH = V // 2
CP = 32           # chunks per streamed piece
NQ = 4            # SWDGE queues: 4 Q7 core-pairs generate gather descriptors concurrently
import os
PHASES = os.environ.get("KPHASES", "12")   # '1' = pass-1 only (debug bisect)
GATHER = os.environ.get("KGATHER", "1") == "1"  # 0: replace gather with dma
GB = 8            # blocks64 staged per y1 DMA
GG = 8            # block128 groups staged per out DMA


def _pad128(n):
    return max((n + 127) & ~127, 128)


def build_plan(rows, cols, vals, split_parity):
    """Chunk tables: slots sorted by dest block64 (and col parity when
    split_parity), padded per group to 128-multiples (>=128 per block).

    Returns dict with per-slot arrays (cols, local row, val) and per-block
    chunk counts. For split_parity, each block's chunks are even-cols chunks
    followed by odd-cols chunks (counts in nch_e / nch_o)."""
    order = np.argsort(rows, kind="stable")
    rows_s, cols_s, vals_s = rows[order], cols[order], vals[order]
    blk = rows_s // W
    out_cols, out_radj, out_val, parities = [], [], [], []
    nch_e = np.zeros(NB, np.int32)
    nch_o = np.zeros(NB, np.int32)
    lo_all = np.searchsorted(blk, np.arange(NB), "left")
    hi_all = np.searchsorted(blk, np.arange(NB), "right")
    for b in range(NB):
        lo, hi = lo_all[b], hi_all[b]
        rb, cb, vb = rows_s[lo:hi] - b * W, cols_s[lo:hi], vals_s[lo:hi]
        groups = []
        if split_parity:
            sel = (cb & 1) == 0
            groups.append((rb[sel], cb[sel], vb[sel], 0))
            groups.append((rb[~sel], cb[~sel], vb[~sel], 1))
        else:
            groups.append((rb, cb, vb, 0))
        for gr, gc, gv, par in groups:
            n = _pad128(len(gr))
            pad = n - len(gr)
            out_radj.append(np.concatenate([gr, np.zeros(pad, gr.dtype)]))
            out_cols.append(np.concatenate([gc, np.full(pad, par, gc.dtype)]))
            out_val.append(np.concatenate([gv, np.zeros(pad, np.float32)]))
            parities.append(np.full(n // 128, par, np.int32))
            if par == 0:
                nch_e[b] = n // 128
            else:
                nch_o[b] = n // 128
    return dict(
        cols=np.concatenate(out_cols),
        radj=np.concatenate(out_radj).astype(np.int32),
        val=np.concatenate(out_val).astype(np.float32),
        nch_e=nch_e, nch_o=nch_o,
        parity=np.concatenate(parities),
    )


def slots_to_chunk_layout(arr):
    """[nslots(, d)] -> [128, nchunks(, d)]: slot j -> [j%128, j//128]."""
    n = arr.shape[0] // 128
    a = arr.reshape(n, 128, *arr.shape[1:])
    return np.ascontiguousarray(np.moveaxis(a, 1, 0))


def wrap_idx16(idx):
    """dma_gather idx layout [128, n/16] int16: idx j at [j%16, j//16],
    replicated across the 8 groups of 16 partitions."""
    n = len(idx)
    assert n % 128 == 0
    w = np.zeros((16, n // 16), np.int16)
    for p in range(16):
        w[p, :] = idx[p::16]
    return np.ascontiguousarray(np.tile(w, (8, 1)))


def make_s(plan):
    """Dense scatter matrices [128, C, W] bf16: S[s, c, r] = val at slot
    (c*128+s) if its local dest row == r else 0."""
    nslots = len(plan["val"])
    C = nslots // 128
    s = np.zeros((C, 128, W), np.float32)
    ci = np.arange(nslots) // 128
    si = np.arange(nslots) % 128
    s[ci, si, plan["radj"]] = plan["val"]
    return np.ascontiguousarray(np.moveaxis(s, 1, 0)).astype(NPBF16)


def build_kernel(nc, C1, C2, n1, n2e, n2o, par2):
    NB128 = V // 128
    g1_d = nc.dram_tensor("g1", [128, C1, F], BF16, kind="ExternalInput")
    s1_d = nc.dram_tensor("s1", [128, C1, W], BF16, kind="ExternalInput")
    s2_d = nc.dram_tensor("s2", [128, C2, W], BF16, kind="ExternalInput")
    idx2_d = nc.dram_tensor("idx2", [128, C2 * 8], dt.int16, kind="ExternalInput")
    x0t_d = nc.dram_tensor("x0t", [F, V], BF16, kind="ExternalInput")
    w_d = nc.dram_tensor("w3", [2 * F, 3, F], BF16, kind="ExternalInput")
    bias_d = nc.dram_tensor("bias", [F, 1], F32, kind="ExternalInput")
    ident_d = nc.dram_tensor("ident", [128, 64], F32, kind="ExternalInput")
    y1_d = nc.dram_tensor("y1", [V, F], BF16)  # internal, row-major
    outt_d = nc.dram_tensor("outt", [F, V], F32, kind="ExternalOutput")

    def pieces(C):
        return [(p * CP, min(CP, C - p * CP)) for p in range((C + CP - 1) // CP)]

    with tile.TileContext(nc) as tc:
        with (
            tc.tile_pool(name="const", bufs=1) as cpool,
            tc.tile_pool(name="ybig", bufs=1) as ypool,
            tc.tile_pool(name="g1p", bufs=3) as g1pool,
            tc.tile_pool(name="s1p", bufs=3) as s1pool,
            tc.tile_pool(name="g2p", bufs=4) as g2pool,
            tc.tile_pool(name="s2p", bufs=4) as s2pool,
            tc.tile_pool(name="idxp", bufs=4) as idxpool,
            tc.tile_pool(name="ztp", bufs=2) as zpool,
            tc.tile_pool(name="xtp", bufs=2) as xtpool,
            tc.tile_pool(name="stg", bufs=3) as stgpool,
            tc.tile_pool(name="t32", bufs=3) as t32pool,
            tc.tile_pool(name="psA", bufs=2, space="PSUM") as psA,
            tc.tile_pool(name="psT", bufs=2, space="PSUM") as psT,
            tc.tile_pool(name="psB", bufs=2, space="PSUM") as psB,
            tc.tile_pool(name="psG", bufs=1, space="PSUM") as psG,
        ):
            ident_t = cpool.tile([128, 64], F32)
            nc.sync.dma_start(ident_t[:], ident_d.ap())
            w_t = cpool.tile([2 * F, 3, F], BF16)
            nc.sync.dma_start(w_t[:], w_d.ap())
            bias_t = cpool.tile([F, 1], F32)
            nc.sync.dma_start(bias_t[:], bias_d.ap())

            y1t_t = ypool.tile([128, VH], BF16)  # p = f + 64*(v >= VH)

            # ---------- PASS 1 ----------
            g1_tiles, s1_tiles = [], []
            p1list = pieces(C1)

            def emit_p1(p):
                c0, w = p1list[p]
                g1_t = g1pool.tile([128, CP, F], BF16, tag="g1")
                nc.sync.dma_start(g1_t[:, 0:w, :], g1_d.ap()[:, c0:c0 + w, :])
                s1_t = s1pool.tile([128, CP, W], BF16, tag="s1")
                nc.sync.dma_start(s1_t[:, 0:w, :], s1_d.ap()[:, c0:c0 + w, :])
                g1_tiles.append(g1_t)
                s1_tiles.append(s1_t)

            cglob = 0
            ystage = None
            for b in range(NB):
                ps = psA.tile([64, W], F32, tag="psA")
                for j in range(n1[b]):
                    while cglob // CP >= len(g1_tiles):
                        emit_p1(len(g1_tiles))
                    p, cip = divmod(cglob, CP)
                    nc.tensor.matmul(ps[:], g1_tiles[p][:, cip, :],
                                     s1_tiles[p][:, cip, :],
                                     start=(j == 0), stop=(j == n1[b] - 1))
                    cglob += 1
                half, off = divmod(b * W, VH)
                ysl = y1t_t[64 * half:64 * half + 64, off:off + W]
                nc.vector.tensor_copy(ysl, ps[:])
                t32 = t32pool.tile([64, W], F32, tag="t32")
                nc.scalar.copy(t32[:], ps[:])
                pt = psT.tile([W, 64], F32, tag="psT")
                nc.tensor.transpose(pt[:], t32[:], ident_t[0:64, :])
                if b % GB == 0:
                    ystage = stgpool.tile([W, GB, 64], BF16, tag="yst")
                nc.scalar.copy(ystage[:, b % GB, :], pt[:])
                if b % GB == GB - 1:
                    g = b // GB
                    dst = y1_d.ap().rearrange("(g e p) f -> g p e f", e=GB, p=W)
                    nc.sync.dma_start(dst[g], ystage[:])

            # ---------- PHASE boundary (debug bisect) ----------
            if '2' not in PHASES:
                ot = outt_d.ap().rearrange("f (h v) -> h f v", h=2)
                for g in range(VH // 512):
                    st = stgpool.tile([128, 512], F32, tag="dbg")
                    nc.vector.tensor_copy(st[:], y1t_t[:, g * 512:(g + 1) * 512])
                    nc.sync.dma_start(ot[0][:, g * 512:(g + 1) * 512], st[0:64, :])
                    nc.sync.dma_start(ot[1][:, g * 512:(g + 1) * 512], st[64:128, :])
                return

            # ---------- PASS 2 ----------
            y1pair = y1_d.ap().rearrange("(p two) f -> p (two f)", two=2)
            p2list = pieces(C2)
            g2_tiles, s2_tiles = [], []

            def emit_p2(p):
                c0, w = p2list[p]
                it = idxpool.tile([128, CP * 8], dt.int16, tag="idx2")
                nc.sync.dma_start(it[:, 0:w * 8],
                                  idx2_d.ap()[:, c0 * 8:(c0 + w) * 8])
                s2_t = s2pool.tile([128, CP, W], BF16, tag="s2")
                nc.sync.dma_start(s2_t[:, 0:w, :], s2_d.ap()[:, c0:c0 + w, :])
                gt = g2pool.tile([128, CP, 2 * F], BF16, tag="g2")
                if not GATHER:
                    nc.sync.dma_start(
                        gt[:, 0:w, :],
                        y1pair.rearrange("(a p) f -> p a f", p=128)[:, 0:w, :])
                    g2_tiles.append(gt)
                    s2_tiles.append(s2_t)
                    return
                nc.gpsimd.dma_gather(
                    gt[:, 0:w, :], y1pair, it[:, 0:w * 8],
                    num_idxs=w * 128, num_idxs_reg=w * 128,
                    elem_size=2 * F, single_packet=False,
                    queue_num=p % NQ,
                )
                g2_tiles.append(gt)
                s2_tiles.append(s2_t)

            cglob = 0
            ostage = None
            xt_t = None
            zt_t = None
            for b in range(NB):
                ps2 = psB.tile([64, W], F32, tag="psB")
                tot = n2e[b] + n2o[b]
                for j in range(tot):
                    while cglob // CP >= len(g2_tiles):
                        emit_p2(len(g2_tiles))
                    p, cip = divmod(cglob, CP)
                    par = par2[cglob]
                    nc.tensor.matmul(
                        ps2[:], g2_tiles[p][:, cip, 64 * par:64 * par + 64],
                        s2_tiles[p][:, cip, :],
                        start=(j == 0), stop=(j == tot - 1))
                    cglob += 1
                if b % 2 == 0:
                    zt_t = zpool.tile([64, 2, W], BF16, tag="zt")
                nc.vector.tensor_copy(zt_t[:, b % 2, :], ps2[:])
                if b % 2 == 1:
                    bb = b // 2  # block128 index
                    half = (bb * 128) // VH
                    off = (bb * 128) % VH
                    if bb % GG == 0:
                        xt_t = xtpool.tile([F, GG, 128], BF16, tag="xt")
                        nc.sync.dma_start(
                            xt_t[:], x0t_d.ap()[:, bb * 128:(bb + GG) * 128]
                            .rearrange("f (e v) -> f e v", e=GG))
                        ostage = stgpool.tile([F, GG, 128], F32, tag="ost")
                    pg = psG.tile([64, 128], F32, tag="psG")
                    nc.tensor.matmul(pg[:], w_t[0:64, 0, :],
                                     xt_t[:, bb % GG, :], start=True, stop=False)
                    nc.tensor.matmul(pg[:], w_t[0:64, 2, :],
                                     zt_t[:].rearrange("f two w -> f (two w)"),
                                     start=False, stop=True)
                    pg2 = psG.tile([64, 128], F32, tag="psG2")
                    nc.tensor.matmul(pg2[:], w_t[64 * half:64 * half + 64, 1, :],
                                     y1t_t[64 * half:64 * half + 64, off:off + 128],
                                     start=True, stop=True)
                    nc.scalar.activation(
                        ostage[:, bb % GG, :], pg[:],
                        mybir.ActivationFunctionType.Identity, bias=bias_t[:])
                    nc.vector.tensor_tensor(
                        ostage[:, bb % GG, :], ostage[:, bb % GG, :], pg2[:],
                        op=mybir.AluOpType.add)
                    if bb % GG == GG - 1:
                        g = bb // GG
                        nc.sync.dma_start(
                            outt_d.ap()[:, g * GG * 128:(g + 1) * GG * 128],
                            ostage[:].rearrange("f e v -> f (e v)"))


def host_prepare(rows, cols, vals):
    """Graph-dependent (core-independent) host tables."""
    plan1 = build_plan(rows, cols, vals, split_parity=False)
    plan2 = build_plan(rows, cols, vals, split_parity=True)
    C1 = len(plan1["val"]) // 128
    C2 = len(plan2["val"]) // 128
    tabs = dict(
        C1=C1, C2=C2,
        n1=plan1["nch_e"], n2e=plan2["nch_e"], n2o=plan2["nch_o"],
        par2=plan2["parity"],
        s1=make_s(plan1), s2=make_s(plan2),
        idx2=wrap_idx16((plan2["cols"] >> 1).astype(np.int16)),
        cols1=plan1["cols"],
        ident=np.tile(np.eye(64, dtype=np.float32), (2, 1)),
    )
    return tabs


def make_in_map(tabs, x_slab, weight, bias):
    """Per-core inputs. x_slab [V, F] f32."""
    g1 = slots_to_chunk_layout(x_slab[tabs["cols1"], :].astype(NPBF16))
    wa = weight[:, 0, :] - weight[:, 2, :]
    wb = weight[:, 1, :]
    wc = 2.0 * weight[:, 2, :]
    w3 = np.ascontiguousarray(np.tile(
        np.stack([wa, wb, wc], axis=1), (2, 1, 1))).astype(NPBF16)
    return {
        "g1": np.ascontiguousarray(g1),
        "s1": tabs["s1"], "s2": tabs["s2"], "idx2": tabs["idx2"],
        "x0t": np.ascontiguousarray(x_slab.T).astype(NPBF16),
        "w3": w3,
        "bias": bias.reshape(F, 1).astype(np.float32),
        "ident": tabs["ident"],
    }


_KERNEL_CACHE = {}


def _get_compiled(tabs):
    key = "k"
    if key not in _KERNEL_CACHE:
        nc = bacc.Bacc("TRN2", target_bir_lowering=False, debug=False,
                       num_devices=8, num_swdge_queues=NQ)
        build_kernel(nc, tabs["C1"], tabs["C2"],
                     tabs["n1"], tabs["n2e"], tabs["n2o"], tabs["par2"])
        nc.compile()
        _KERNEL_CACHE[key] = nc
    return _KERNEL_CACHE[key]


def kernel(inputs, weight, bias, lap_rows, lap_cols, lap_vals):
    from concourse.bass_utils import run_bass_kernel_spmd

    B, Vi, Fi = inputs.shape
    assert (B, Vi, Fi) == (8, V, F)
    rows = np.asarray(lap_rows).astype(np.int64)
    cols = np.asarray(lap_cols).astype(np.int64)
    vals = np.asarray(lap_vals).astype(np.float32)
    inputs = np.asarray(inputs, dtype=np.float32)
    weight = np.asarray(weight, dtype=np.float32)
    bias = np.asarray(bias, dtype=np.float32)

    tabs = host_prepare(rows, cols, vals)
    nc = _get_compiled(tabs)

    in_maps = [make_in_map(tabs, inputs[c], weight, bias) for c in range(B)]
    res = run_bass_kernel_spmd(nc, in_maps, list(range(B)))
    out = np.stack([np.ascontiguousarray(res.results[c]["outt"].T)
                    for c in range(B)])
    return out.astype(np.float32)
